# revision 1
# baseline (speedup 1.0000x reference)
"""GQA attention kernel for Trainium2, 8 NeuronCores.

Sharding: data-parallel over batch (B=2) x tensor-parallel over KV heads
(HKV=4) -> 8 cores.  Core c handles batch b=c//4, kv-head j=c%4 with its
G=4 query heads.  out_proj is row-parallel; partials are reduced on host.

Layout strategy: everything transposed ([feature, seq]) so that
projections, scores and PV matmuls all consume operands natively:
  qT/kT from proj (lhsT=W, rhs=hiddenT),
  scoresT[j,i] (lhsT=kT chunk, rhs=qT),
  PV (lhsT=v natural [s,d] with a ones column -> softmax denominator),
  out_projT (lhsT=oT, rhs=Wo rows).
Softmax skips max-subtraction: q,k are rmsnorm'd so |q.k/8| <= 8 and
exp() is safe in fp32 for any non-positive mask.
"""

import numpy as np
import ml_dtypes

import concourse.bacc as bacc
import concourse.mybir as mybir
from concourse import bass_isa
from concourse.tile import TileContext
from concourse.bass_utils import run_bass_kernel_spmd

BF16 = mybir.dt.bfloat16
F32 = mybir.dt.float32
AL = mybir.AluOpType

B, S, HID = 2, 2048, 1024
H, HKV, D = 16, 4, 64
G = H // HKV  # 4 query heads per kv head
QSEL = 2 * G * D  # 512: own 256 cols + rope-partner 256 cols
ROPE_BASE = 10000.0
EPS = float(np.finfo(np.float32).eps)

NB = ml_dtypes.bfloat16

_cache: dict = {}


def _build(use_mask: bool, debug: bool = False):
    nc = bacc.Bacc("TRN2", target_bir_lowering=False)

    hT = nc.dram_tensor("hT", [8, 128, S], BF16, kind="ExternalInput")
    wq = nc.dram_tensor("wq", [8, 128, QSEL], BF16, kind="ExternalInput")
    wk = nc.dram_tensor("wk", [8, 128, 128], BF16, kind="ExternalInput")
    wv = nc.dram_tensor("wv", [8, 128, 64], BF16, kind="ExternalInput")
    wo = nc.dram_tensor("wo", [2, 128, HID], BF16, kind="ExternalInput")
    qco = nc.dram_tensor("qco", [2, 128, S], BF16, kind="ExternalInput")
    qsi = nc.dram_tensor("qsi", [2, 128, S], BF16, kind="ExternalInput")
    kco = nc.dram_tensor("kco", [64, S], BF16, kind="ExternalInput")
    ksi = nc.dram_tensor("ksi", [64, S], BF16, kind="ExternalInput")
    bqv = nc.dram_tensor("bqv", [4, 128, 1], F32, kind="ExternalInput")
    bkv = nc.dram_tensor("bkv", [2, 64, 1], F32, kind="ExternalInput")
    bvv = nc.dram_tensor("bvv", [128, 1], F32, kind="ExternalInput")
    mk = (
        nc.dram_tensor("mk", [16, 128, S], F32, kind="ExternalInput")
        if use_mask
        else None
    )
    y = nc.dram_tensor("y", [16, 128, HID], F32, kind="ExternalOutput")
    if debug:
        d_qr = nc.dram_tensor("d_qr", [128, 2, S], BF16, kind="ExternalOutput")
        d_qn = nc.dram_tensor("d_qn", [128, 2, S], BF16, kind="ExternalOutput")
        d_kn = nc.dram_tensor("d_kn", [128, S], BF16, kind="ExternalOutput")
        d_va = nc.dram_tensor("d_va", [128, 16, 66], BF16, kind="ExternalOutput")
        d_pt = nc.dram_tensor("d_pt", [128, 4, 512], BF16, kind="ExternalOutput")
        d_on = nc.dram_tensor("d_on", [128, 2, S], BF16, kind="ExternalOutput")

    with TileContext(nc) as tc:
        with tc.tile_pool(name="const", bufs=1) as cp:
            # ---- persistent tiles -------------------------------------
            wo_sb = cp.tile([128, 2, HID], BF16)
            for cc in range(2):
                nc.sync.dma_start(out=wo_sb[:, cc, :], in_=wo[cc])
            bv_sb = cp.tile([128, 1], F32)
            nc.sync.dma_start(out=bv_sb[:], in_=bvv[:])

            qr = cp.tile([128, 2, S], BF16)   # rope'd q (own 256 rows)
            qn = cp.tile([128, 2, S], BF16)   # rmsnorm'd q
            kr = cp.tile([64, S], BF16)
            kn2 = cp.tile([128, S], BF16)     # rmsnorm'd k, duplicated rows
            v_all = cp.tile([128, 16, 66], BF16)  # v natural + ones col
            oTn = cp.tile([128, 2, S], BF16)  # normalized attn out (oT)
            eps_sb = cp.tile([128, 1], F32)
            nc.vector.memset(eps_sb[:], EPS)

            # ---- phase 1: projections + rope (pools close after) ------
            with (
                tc.tile_pool(name="projc", bufs=1) as pjc,
                tc.tile_pool(name="ropetmp", bufs=4) as rtp,
                tc.tile_pool(name="stats", bufs=1) as stp,
                tc.tile_pool(name="pproj", bufs=1, space="PSUM") as pp,
            ):
                hT_sb = pjc.tile([128, 8, S], BF16)
                for ko in range(8):
                    nc.sync.dma_start(out=hT_sb[:, ko, :], in_=hT[ko])
                wq_sb = pjc.tile([128, 8, QSEL], BF16)
                wk_sb = pjc.tile([128, 8, 128], BF16)
                wv_sb = pjc.tile([128, 8, 64], BF16)
                for ko in range(8):
                    nc.sync.dma_start(out=wq_sb[:, ko, :], in_=wq[ko])
                    nc.sync.dma_start(out=wk_sb[:, ko, :], in_=wk[ko])
                    nc.sync.dma_start(out=wv_sb[:, ko, :], in_=wv[ko])
                qco_sb = pjc.tile([128, 2, S], BF16)
                qsi_sb = pjc.tile([128, 2, S], BF16)
                for cc in range(2):
                    nc.sync.dma_start(out=qco_sb[:, cc, :], in_=qco[cc])
                    nc.sync.dma_start(out=qsi_sb[:, cc, :], in_=qsi[cc])
                kco_sb = pjc.tile([64, S], BF16)
                ksi_sb = pjc.tile([64, S], BF16)
                nc.sync.dma_start(out=kco_sb[:], in_=kco[:])
                nc.sync.dma_start(out=ksi_sb[:], in_=ksi[:])
                bq_sb = pjc.tile([128, 4, 1], F32)
                for co in range(4):
                    nc.sync.dma_start(out=bq_sb[:, co, :], in_=bqv[co])
                bk_sb = pjc.tile([64, 2, 1], F32)
                for t in range(2):
                    nc.sync.dma_start(out=bk_sb[:, t, :], in_=bkv[t])
                for so in range(4):
                    sl = slice(so * 512, (so + 1) * 512)
                    psq = pp.tile([128, 4, 512], F32, tag="psq")
                    psk = pp.tile([64, 2, 512], F32, tag="psk")
                    for ko in range(8):
                        st, sp = ko == 0, ko == 7
                        for co in range(4):
                            nc.tensor.matmul(
                                psq[:, co, :],
                                lhsT=wq_sb[:, ko, co * 128:(co + 1) * 128],
                                rhs=hT_sb[:, ko, sl],
                                start=st, stop=sp,
                            )
                        for t in range(2):
                            nc.tensor.matmul(
                                psk[:, t, :],
                                lhsT=wk_sb[:, ko, t * 64:(t + 1) * 64],
                                rhs=hT_sb[:, ko, sl],
                                start=st, stop=sp,
                            )
                    # rope: roped = (own + b_own)*cos + (partner + b_par)*sin
                    for co in range(2):
                        t1 = rtp.tile([128, 512], F32, tag="t1")
                        t2 = rtp.tile([128, 512], F32, tag="t2")
                        nc.vector.scalar_tensor_tensor(
                            t1, psq[:, co, :], bq_sb[:, co, :],
                            qco_sb[:, co, sl], AL.add, AL.mult,
                        )
                        nc.vector.scalar_tensor_tensor(
                            t2, psq[:, co + 2, :], bq_sb[:, co + 2, :],
                            qsi_sb[:, co, sl], AL.add, AL.mult,
                        )
                        nc.vector.tensor_tensor(qr[:, co, sl], t1, t2, AL.add)
                    t1 = rtp.tile([64, 512], F32, tag="t1k")
                    t2 = rtp.tile([64, 512], F32, tag="t2k")
                    nc.vector.scalar_tensor_tensor(
                        t1, psk[:, 0, :], bk_sb[:, 0, :],
                        kco_sb[:, sl], AL.add, AL.mult,
                    )
                    nc.vector.scalar_tensor_tensor(
                        t2, psk[:, 1, :], bk_sb[:, 1, :],
                        ksi_sb[:, sl], AL.add, AL.mult,
                    )
                    nc.vector.tensor_tensor(kr[:, sl], t1, t2, AL.add)

                # v projection (natural layout) + ones column
                for sc in range(16):
                    psv = pp.tile([128, 64], F32, tag="psv")
                    for ko in range(8):
                        nc.tensor.matmul(
                            psv[:],
                            lhsT=hT_sb[:, ko, sc * 128:(sc + 1) * 128],
                            rhs=wv_sb[:, ko, :],
                            start=(ko == 0), stop=(ko == 7),
                        )
                    nc.vector.tensor_copy(v_all[:, sc, 0:64], psv[:])
                    nc.vector.memset(v_all[:, sc, 64:65], 1.0)

                # ---- phase 2: rmsnorm over each head's 64 dims ------------
                # all ops partition-aligned; the two heads sharing a 128-row
                # chunk are reduced separately (channels=64) but share the
                # sqrt/recip/mul ops.
                # gpsimd partition ops only behave at partition base 0 on HW:
                # odd-parity rows are DMA-shifted to base 0 for the reduce and
                # the resulting scale is DMA-shifted back up.
                def rmsnorm(s_in, s_out, npart):
                    sq = stp.tile([128, S], F32, tag="sq", name="sq")[:npart]
                    rn = stp.tile([128, S], F32, tag="rn", name="rn")[:npart]
                    nc.vector.tensor_tensor(sq, s_in, s_in, AL.mult)
                    ss_e = stp.tile([64, S], F32, tag="ss_e", name="ss_e")
                    nc.gpsimd.partition_all_reduce(
                        ss_e, sq[0:64, :], channels=64,
                        reduce_op=bass_isa.ReduceOp.add,
                    )
                    nc.scalar.activation(
                        rn[0:64, :], ss_e, mybir.ActivationFunctionType.Sqrt,
                        bias=eps_sb[0:64], scale=1.0 / 64.0,
                    )
                    nc.vector.reciprocal(rn[0:64, :], rn[0:64, :])
                    if npart == 128:
                        sq2 = stp.tile([64, S], F32, tag="sq2", name="sq2")
                        nc.sync.dma_start(out=sq2, in_=sq[64:128, :])
                        ss_o = stp.tile([64, S], F32, tag="ss_o", name="ss_o")
                        nc.gpsimd.partition_all_reduce(
                            ss_o, sq2, channels=64,
                            reduce_op=bass_isa.ReduceOp.add,
                        )
                        rno = stp.tile([64, S], F32, tag="rno", name="rno")
                        nc.scalar.activation(
                            rno, ss_o, mybir.ActivationFunctionType.Sqrt,
                            bias=eps_sb[0:64], scale=1.0 / 64.0,
                        )
                        nc.vector.reciprocal(rno, rno)
                        nc.sync.dma_start(out=rn[64:128, :], in_=rno)
                    nc.vector.tensor_tensor(s_out, s_in, rn, AL.mult)

                for ch in range(2):
                    rmsnorm(qr[:, ch, :], qn[:, ch, :], 128)
                rmsnorm(kr[:], kn2[0:64, :], 64)
                # duplicate kn rows so scores lhsT base can match qn rows
                nc.sync.dma_start(out=kn2[64:128, :], in_=kn2[0:64, :])
                if debug:
                    nc.sync.dma_start(out=d_qr[:], in_=qr[:])
                    nc.sync.dma_start(out=d_qn[:], in_=qn[:])
                    nc.sync.dma_start(out=d_kn[:], in_=kn2[:])
                    nc.sync.dma_start(out=d_va[:], in_=v_all[:])

            with (
                tc.tile_pool(name="probs", bufs=3) as ppool,
                tc.tile_pool(name="bcast", bufs=1) as bcp,
                tc.tile_pool(name="ysb", bufs=2) as ypool,
                tc.tile_pool(name="mtile", bufs=3) as mpool,
            ):
                # ---- phase 3: attention (flash over i-chunks) -------------
                with (
                    tc.tile_pool(name="pscore", bufs=2, space="PSUM") as psc,
                    tc.tile_pool(name="pacc", bufs=1, space="PSUM") as pac,
                ):
                    for ic in range(4):
                        isl = slice(ic * 512, (ic + 1) * 512)
                        pso = pac.tile([65, 4, 512], F32, tag="pso")
                        for jc in range(16):
                            pT = ppool.tile([128, 4, 512], BF16, tag="pT")
                            if use_mask:
                                mkt = mpool.tile([128, 512], F32, tag="mkt")
                                nc.sync.dma_start(out=mkt[:], in_=mk[jc][:, isl])
                            for pair in range(2):
                                pss = psc.tile([128, 2, 512], F32, tag="pss")
                                for hh in range(2):
                                    hd = pair * 2 + hh
                                    qrows = slice(64 * (hd % 2), 64 * (hd % 2) + 64)
                                    nc.tensor.matmul(
                                        pss[:, hh, :],
                                        lhsT=kn2[qrows, jc * 128:(jc + 1) * 128],
                                        rhs=qn[qrows, hd // 2, isl],
                                        start=True, stop=True,
                                    )
                                if use_mask:
                                    sm = mpool.tile([128, 2, 512], F32, tag="sm")
                                    nc.vector.scalar_tensor_tensor(
                                        sm, pss[:], 0.125,
                                        mkt[:, None, :].to_broadcast((128, 2, 512)),
                                        AL.mult, AL.add,
                                    )
                                    nc.scalar.activation(
                                        pT[:, pair * 2:pair * 2 + 2, :], sm,
                                        mybir.ActivationFunctionType.Exp,
                                    )
                                else:
                                    nc.scalar.activation(
                                        pT[:, pair * 2:pair * 2 + 2, :], pss,
                                        mybir.ActivationFunctionType.Exp,
                                        scale=0.125,
                                    )
                            if debug and ic == 0 and jc == 0:
                                nc.sync.dma_start(out=d_pt[:], in_=pT[:])
                            for hd in range(4):
                                nc.tensor.matmul(
                                    pso[:, hd, :],
                                    lhsT=v_all[:, jc, 0:65],
                                    rhs=pT[:, hd, :],
                                    start=(jc == 0), stop=(jc == 15),
                                )
                        # normalize: recip of denominator row, broadcast over
                        # the 64 head dims, multiply.  odd heads are written at
                        # partition base 0 then DMA'd to rows 64-127.
                        rcp = bcp.tile([65, 4, 512], F32, tag="rcp")
                        nc.vector.reciprocal(rcp[64:65, :, :], pso[64:65, :, :])
                        rcp0 = bcp.tile([1, 4, 512], F32, tag="rcp0")
                        nc.sync.dma_start(out=rcp0, in_=rcp[64:65, :, :])
                        for hd in range(4):
                            rb = bcp.tile([64, 512], F32, tag="rb")
                            nc.gpsimd.partition_broadcast(
                                rb, rcp0[0:1, hd, :], channels=64
                            )
                            if hd % 2 == 0:
                                nc.vector.tensor_tensor(
                                    oTn[0:64, hd // 2, isl], pso[0:64, hd, :], rb, AL.mult
                                )
                            else:
                                ood = bcp.tile([64, 512], BF16, tag="ood")
                                nc.vector.tensor_tensor(
                                    ood, pso[0:64, hd, :], rb, AL.mult
                                )
                                nc.sync.dma_start(
                                    out=oTn[64:128, hd // 2, isl], in_=ood
                                )

                # ---- phase 4: v-projection bias (zero in practice) --------
                for ch in range(2):
                    nc.vector.tensor_scalar_add(oTn[:, ch, :], oTn[:, ch, :], bv_sb[:])
                if debug:
                    nc.sync.dma_start(out=d_on[:], in_=oTn[:])

                # ---- phase 5: out_proj (row-parallel partial) -------------
                with tc.tile_pool(name="py", bufs=2, space="PSUM") as pyp:
                    for sc in range(16):
                        ssl = slice(sc * 128, (sc + 1) * 128)
                        y_sb = ypool.tile([128, HID], F32, tag="ysb")
                        for ec in range(2):
                            psy = pyp.tile([128, 512], F32, tag="psy")
                            for cc in range(2):
                                nc.tensor.matmul(
                                    psy[:],
                                    lhsT=oTn[:, cc, ssl],
                                    rhs=wo_sb[:, cc, ec * 512:(ec + 1) * 512],
                                    start=(cc == 0), stop=(cc == 1),
                                )
                            if ec == 0:
                                nc.scalar.copy(y_sb[:, 0:512], psy[:])
                            else:
                                nc.vector.tensor_copy(y_sb[:, 512:1024], psy[:])
                        nc.sync.dma_start(out=y[sc], in_=y_sb[:])

    nc.compile()
    return nc


def _get(use_mask: bool):
    if use_mask not in _cache:
        _cache[use_mask] = _build(use_mask)
    return _cache[use_mask]


def _host_prep(hidden_state, attention_mask, Wq, bq, Wk, bk, Wv, bv, Wo, use_mask):
    """Build the 8 per-core input maps."""
    half_q, half_k = HID // 2, (HKV * D) // 2  # 512, 128
    inv_q = ROPE_BASE ** (-np.arange(half_q, dtype=np.float64) / half_q)
    inv_k = ROPE_BASE ** (-np.arange(half_k, dtype=np.float64) / half_k)
    s_idx = np.arange(S, dtype=np.float64)
    ang_q = inv_q[:, None] * s_idx[None, :]  # [512, S] freq-major
    ang_k = inv_k[:, None] * s_idx[None, :]  # [128, S]
    cos_q, sin_q = np.cos(ang_q), np.sin(ang_q)
    cos_k, sin_k = np.cos(ang_k), np.sin(ang_k)

    in_maps = []
    for core in range(8):
        b, j = core // 4, core % 4
        own_q = np.arange(j * 256, (j + 1) * 256)
        par_q = own_q + 512 if j < 2 else own_q - 512
        fidx_q = own_q if j < 2 else own_q - 512
        sign = -1.0 if j < 2 else 1.0
        own_k = np.arange(j * 64, (j + 1) * 64)
        par_k = own_k + 128 if j < 2 else own_k - 128
        fidx_k = own_k if j < 2 else own_k - 128

        hTc = np.ascontiguousarray(hidden_state[b].T).astype(NB).reshape(8, 128, S)
        wq_c = np.concatenate([Wq[:, own_q], Wq[:, par_q]], axis=1)
        wq_c = wq_c.astype(NB).reshape(8, 128, QSEL)
        wk_c = np.concatenate([Wk[:, own_k], Wk[:, par_k]], axis=1)
        wk_c = wk_c.astype(NB).reshape(8, 128, 128)
        wv_c = Wv[:, own_k].astype(NB).reshape(8, 128, 64)
        wo_c = Wo[j * 256:(j + 1) * 256, :].astype(NB).reshape(2, 128, HID)
        qco_c = cos_q[fidx_q].astype(NB).reshape(2, 128, S)
        qsi_c = (sign * sin_q[fidx_q]).astype(NB).reshape(2, 128, S)
        kco_c = cos_k[fidx_k].astype(NB)
        ksi_c = (sign * sin_k[fidx_k]).astype(NB)
        bq_c = np.concatenate([bq[own_q], bq[par_q]]).astype(np.float32)
        bq_c = bq_c.reshape(4, 128, 1)
        bk_c = np.concatenate([bk[own_k], bk[par_k]]).astype(np.float32)
        bk_c = bk_c.reshape(2, 64, 1)
        bv_c = np.tile(bv[own_k], 2).astype(np.float32).reshape(128, 1)

        m = {
            "hT": hTc, "wq": wq_c, "wk": wk_c, "wv": wv_c, "wo": wo_c,
            "qco": qco_c, "qsi": qsi_c, "kco": kco_c, "ksi": ksi_c,
            "bqv": bq_c, "bkv": bk_c, "bvv": bv_c,
        }
        if use_mask:
            mT = np.ascontiguousarray(attention_mask[b].T).astype(np.float32)
            m["mk"] = mT.reshape(16, 128, S)
        in_maps.append(m)
    return in_maps


def kernel(hidden_state, attention_mask, Wq, bq, Wk, bk, Wv, bv, Wo, bo):
    hidden_state = np.asarray(hidden_state, dtype=np.float32)
    attention_mask = np.asarray(attention_mask, dtype=np.float32)
    use_mask = bool(np.any(attention_mask))
    nc = _get(use_mask)
    in_maps = _host_prep(
        hidden_state, attention_mask,
        np.asarray(Wq, np.float32), np.asarray(bq, np.float32),
        np.asarray(Wk, np.float32), np.asarray(bk, np.float32),
        np.asarray(Wv, np.float32), np.asarray(bv, np.float32),
        np.asarray(Wo, np.float32), use_mask,
    )
    res = run_bass_kernel_spmd(nc, in_maps, list(range(8)))
    out = np.zeros((B, S, HID), dtype=np.float32)
    for core in range(8):
        out[core // 4] += res.results[core]["y"].reshape(S, HID)
    out += np.asarray(bo, np.float32)[None, None, :]
    return out



# revision 5
# speedup vs baseline: 1.4017x; 1.4017x over previous
"""GQA attention kernel for Trainium2, 8 NeuronCores.

Sharding: data-parallel over batch (B=2) x tensor-parallel over KV heads
(HKV=4) -> 8 cores.  Core c handles batch b=c//4, kv-head j=c%4 with its
G=4 query heads.  out_proj is row-parallel; partials are reduced on host.

Layout strategy (v2):
  - Projections in NATURAL orientation (out[seq, feat]): lhsT = hiddenT
    chunk, rhs = W chunk.  RoPE and rmsnorm then operate along the free
    dim (cheap DVE/Pool ops, no partition reduductions).
  - qT / kT for the scores matmuls are produced by DMA-transpose
    (crossbar) instructions; kT's row-64..127 duplicate is folded into
    the same transpose by duplicating kn columns beforehand.
  - scoresT[key, q] = kT^T @ qT per head, exp on ACT (the hard floor:
    ~110us of exp at 0.833 ns/elem), probabilities pT kept in SBUF for a
    full 512-q block.
  - PV in flipped orientation: out[q, d+1] with lhsT = pT chunk,
    rhs = v (with ones column -> denominator lands as column 64).  N=65
    per matmul instead of 512 -> half the PE rows of the baseline.
    Normalization is a per-partition reciprocal + broadcast multiply.
  - oT via DMA-transpose feeds a row-parallel out_proj; partials DMA'd
    per 128-row chunk.
PSUM budget (8 banks): pq 1 | scoresA 2 | scoresB 2 | oraw 2 | y 1.
"""

import numpy as np
import ml_dtypes

import concourse.bacc as bacc
import concourse.mybir as mybir
from concourse.tile import TileContext

BF16 = mybir.dt.bfloat16
F32 = mybir.dt.float32
AL = mybir.AluOpType
AF = mybir.ActivationFunctionType
AX = mybir.AxisListType

B, S, HID = 2, 2048, 1024
H, HKV, D = 16, 4, 64
G = H // HKV          # 4 query heads per kv head
QSEL = 2 * G * D      # 512: own 256 cols + rope-partner 256 cols
ROPE_BASE = 10000.0
EPS = float(np.finfo(np.float32).eps)
NSC = S // 128        # 16 seq chunks
NIC = 4               # 512-wide q blocks

NB = ml_dtypes.bfloat16

_cache: dict = {}


def _build(use_mask: bool, use_bias: bool):
    nc = bacc.Bacc("TRN2", target_bir_lowering=False)

    hT = nc.dram_tensor("hT", [8, 128, S], BF16, kind="ExternalInput")
    wq = nc.dram_tensor("wq", [8, 128, QSEL], BF16, kind="ExternalInput")
    wk = nc.dram_tensor("wk", [8, 128, 128], BF16, kind="ExternalInput")
    wv = nc.dram_tensor("wv", [8, 128, 64], BF16, kind="ExternalInput")
    wo = nc.dram_tensor("wo", [2, 128, HID], BF16, kind="ExternalInput")
    csq = nc.dram_tensor("csq", [NSC, 128, 2, 256], BF16, kind="ExternalInput")
    csk = nc.dram_tensor("csk", [NSC, 128, 2, 64], BF16, kind="ExternalInput")
    y = nc.dram_tensor("y", [NSC, 128, HID], F32, kind="ExternalOutput")
    mk = (
        nc.dram_tensor("mk", [NSC, 128, S], F32, kind="ExternalInput")
        if use_mask
        else None
    )
    if use_bias:
        brq = nc.dram_tensor("brq", [1, QSEL], BF16, kind="ExternalInput")
        brk = nc.dram_tensor("brk", [1, 128], BF16, kind="ExternalInput")
        brv = nc.dram_tensor("brv", [1, 64], BF16, kind="ExternalInput")

    with TileContext(nc) as tc:
        with (
            tc.tile_pool(name="const", bufs=1) as cp,
            tc.tile_pool(name="proj", bufs=1) as pj,
            tc.tile_pool(name="rt", bufs=3) as rt,
            tc.tile_pool(name="stat", bufs=4) as stp,
            tc.tile_pool(name="pT", bufs=20) as ptp,
            tc.tile_pool(name="onat", bufs=3) as onp_,
            tc.tile_pool(name="oTp", bufs=3) as otp,
            tc.tile_pool(name="ysb", bufs=2) as yp,
            tc.tile_pool(name="maskp", bufs=3) as mp,
            tc.tile_pool(name="ps", bufs=1, space="PSUM") as ps,
        ):
            # ---- persistent tiles ------------------------------------
            wo_sb = cp.tile([128, 2, HID], BF16)
            for cc in range(2):
                nc.sync.dma_start(out=wo_sb[:, cc, :], in_=wo[cc])
            v_sb = cp.tile([128, NSC, 66], BF16)
            nc.vector.memset(v_sb[:, :, 64:65], 1.0)
            eps_sb = cp.tile([128, 1], F32)
            nc.vector.memset(eps_sb[:], EPS)
            qT = cp.tile([128, 2, S], BF16)
            kT = cp.tile([128, S], BF16)

            # ---- projection-phase constants --------------------------
            hT_sb = pj.tile([128, 8, S], BF16)
            for ko in range(8):
                nc.sync.dma_start(out=hT_sb[:, ko, :], in_=hT[ko])
            wk_sb = pj.tile([128, 8, 128], BF16)
            nc.sync.dma_start(out=wk_sb[:], in_=wk[:].rearrange("a b c -> b a c"))
            wv_sb = pj.tile([128, 8, 64], BF16)
            nc.sync.dma_start(out=wv_sb[:], in_=wv[:].rearrange("a b c -> b a c"))
            csk_sb = pj.tile([128, NSC, 2, 64], BF16)
            nc.sync.dma_start(out=csk_sb[:], in_=csk[:].rearrange("a b c d -> b a c d"))
            wq_sb = pj.tile([128, 8, QSEL], BF16)
            nc.sync.dma_start(out=wq_sb[:], in_=wq[:].rearrange("a b c -> b a c"))
            csq_sb = pj.tile([128, NSC, 2, 256], BF16)
            nc.sync.dma_start(out=csq_sb[:], in_=csq[:].rearrange("a b c d -> b a c d"))
            if use_bias:
                ones1 = cp.tile([1, 128], BF16)
                nc.vector.memset(ones1[:], 1.0)
                brq_sb = cp.tile([1, QSEL], BF16)
                nc.sync.dma_start(out=brq_sb[:], in_=brq[:])
                brk_sb = cp.tile([1, 128], BF16)
                nc.sync.dma_start(out=brk_sb[:], in_=brk[:])
                brv_sb = cp.tile([1, 64], BF16)
                nc.sync.dma_start(out=brv_sb[:], in_=brv[:])

            def kv_chain(sc):
                ssl = slice(sc * 128, (sc + 1) * 128)
                pk = ps.tile([128, 2, 64], F32, tag="scA")
                pv = ps.tile([128, 64], F32, tag="scB")
                for ko in range(8):
                    st, sp = ko == 0, (ko == 7 and not use_bias)
                    nc.tensor.matmul(
                        pk[:], lhsT=hT_sb[:, ko, ssl], rhs=wk_sb[:, ko, :],
                        start=st, stop=sp,
                    )
                for ko in range(8):
                    st, sp = ko == 0, (ko == 7 and not use_bias)
                    nc.tensor.matmul(
                        pv[:], lhsT=hT_sb[:, ko, ssl], rhs=wv_sb[:, ko, :],
                        start=st, stop=sp,
                    )
                if use_bias:
                    nc.tensor.matmul(pk[:], lhsT=ones1[:], rhs=brk_sb[:],
                                     start=False, stop=True)
                    nc.tensor.matmul(pv[:], lhsT=ones1[:], rhs=brv_sb[:],
                                     start=False, stop=True)
                # rope: kro = raw_own*cos + raw_par*ssin
                t12k = rt.tile([128, 2, 64], BF16, tag="t12k")
                nc.vector.tensor_tensor(t12k[:], pk[:], csk_sb[:, sc, :, :], AL.mult)
                kro = rt.tile([128, 64], BF16, tag="kro")
                nc.gpsimd.tensor_tensor(kro[:], t12k[:, 0, :], t12k[:, 1, :], AL.add)
                sqk = rt.tile([128, 64], BF16, tag="sqk")
                nc.gpsimd.tensor_tensor(sqk[:], kro[:], kro[:], AL.mult)
                rmk = stp.tile([128, 1], F32, tag="rmk")
                nc.vector.tensor_reduce(rmk[:], sqk[:], AX.X, AL.add)
                rsk = stp.tile([128, 1], F32, tag="rsk")
                nc.scalar.activation(rsk[:], rmk[:], AF.Sqrt, bias=eps_sb[:],
                                     scale=1.0 / 64.0)
                rck = stp.tile([128, 1], F32, tag="rck")
                nc.vector.reciprocal(rck[:], rsk[:])
                kn2 = rt.tile([128, 2, 64], BF16, tag="kn2")
                nc.vector.tensor_scalar_mul(kn2[:, 0, :], kro[:], rck[:])
                nc.gpsimd.tensor_copy(kn2[:, 1, :], kn2[:, 0, :])
                nc.sync.dma_start_transpose(out=kT[:, ssl], in_=kn2[:])
                # v natural + copy to sbuf
                nc.vector.tensor_copy(v_sb[:, sc, 0:64], pv[:])

            def q_chain(sc):
                ssl = slice(sc * 128, (sc + 1) * 128)
                pq = ps.tile([128, 2, 256], F32, tag="pq")
                for ko in range(8):
                    st, sp = ko == 0, (ko == 7 and not use_bias)
                    nc.tensor.matmul(
                        pq[:], lhsT=hT_sb[:, ko, ssl], rhs=wq_sb[:, ko, :],
                        start=st, stop=sp,
                    )
                if use_bias:
                    nc.tensor.matmul(pq[:], lhsT=ones1[:], rhs=brq_sb[:],
                                     start=False, stop=True)
                t12 = rt.tile([128, 2, 256], BF16, tag="t12")
                nc.vector.tensor_tensor(t12[:], pq[:], csq_sb[:, sc, :, :], AL.mult)
                qro = rt.tile([128, 256], BF16, tag="qro")
                nc.gpsimd.tensor_tensor(qro[:], t12[:, 0, :], t12[:, 1, :], AL.add)
                sqq = rt.tile([128, 4, 64], BF16, tag="sqq")
                nc.gpsimd.tensor_tensor(
                    sqq[:], qro[:].rearrange("p (h d) -> p h d", h=4), qro[:].rearrange("p (h d) -> p h d", h=4), AL.mult
                )
                rms = stp.tile([128, 4], F32, tag="rms")
                nc.vector.tensor_reduce(rms[:], sqq[:], AX.X, AL.add)
                rsq = stp.tile([128, 4], F32, tag="rsq")
                nc.scalar.activation(rsq[:], rms[:], AF.Sqrt, bias=eps_sb[:],
                                     scale=1.0 / 64.0)
                rcq = stp.tile([128, 4], F32, tag="rcq")
                nc.vector.reciprocal(rcq[:], rsq[:])
                qn = rt.tile([128, 4, 64], BF16, tag="qn")
                nc.vector.tensor_tensor(
                    qn[:], qro[:].rearrange("p (h d) -> p h d", h=4),
                    rcq[:, :, None].to_broadcast((128, 4, 64)), AL.mult,
                )
                nc.sync.dma_start_transpose(out=qT[:, :, ssl], in_=qn[:])

            def finish_half(ic, h, oraw):
                # oraw: [128, 8, 128] psum, slices (s2, hd) at s2*4+hd, col 64 = denom
                rcp = stp.tile([128, 8], F32, tag="rcp")
                nc.vector.reciprocal(rcp[:], oraw[:, :, 64:65])
                for s2 in range(2):
                    sub = 2 * h + s2
                    onat = onp_.tile([128, 4, 64], BF16, tag="onat")
                    nc.vector.tensor_tensor(
                        onat[:], oraw[:, s2 * 4:(s2 + 1) * 4, 0:64],
                        rcp[:, s2 * 4:(s2 + 1) * 4, None].to_broadcast((128, 4, 64)),
                        AL.mult,
                    )
                    oTt = otp.tile([128, 2, 128], BF16, tag="oTt")
                    nc.sync.dma_start_transpose(out=oTt[:], in_=onat[:])
                    ysb = yp.tile([128, HID], F32, tag="ysb")
                    for ec in range(2):
                        py = ps.tile([128, 512], F32, tag="py")
                        for cc in range(2):
                            nc.tensor.matmul(
                                py[:], lhsT=oTt[:, cc, :],
                                rhs=wo_sb[:, cc, ec * 512:(ec + 1) * 512],
                                start=(cc == 0), stop=(cc == 1),
                            )
                        nc.vector.tensor_copy(ysb[:, ec * 512:(ec + 1) * 512], py[:])
                    nc.sync.dma_start(out=y[ic * 4 + sub], in_=ysb[:])

            # ---- lead-in: k/v for all chunks, then q for ic 0 --------
            for sc in range(NSC):
                kv_chain(sc)
            for sc in range(4):
                q_chain(sc)

            # ---- attention -------------------------------------------
            for ic in range(NIC):
                isl = slice(ic * 512, (ic + 1) * 512)
                pts = []
                oraw0 = ps.tile([128, 8, 128], F32, tag="oraw")
                for jc in range(16):
                    pT_t = ptp.tile([128, 4, 512], BF16, tag="pT")
                    pts.append(pT_t)
                    if use_mask:
                        mkt = mp.tile([128, 512], F32, tag="mkt")
                        nc.sync.dma_start(out=mkt[:], in_=mk[jc][:, isl])
                    for pair in range(2):
                        pss = ps.tile([128, 2, 512], F32, tag=("scA" if pair == 0 else "scB"))
                        for hh in range(2):
                            rows = slice(64 * hh, 64 * hh + 64)
                            nc.tensor.matmul(
                                pss[:, hh, :],
                                lhsT=kT[rows, jc * 128:(jc + 1) * 128],
                                rhs=qT[rows, pair, isl],
                                start=True, stop=True,
                            )
                        if use_mask:
                            sm = mp.tile([128, 2, 512], F32, tag="sm")
                            nc.vector.scalar_tensor_tensor(
                                sm[:], pss[:], 0.125,
                                mkt[:, None, :].to_broadcast((128, 2, 512)),
                                AL.mult, AL.add,
                            )
                            nc.scalar.activation(
                                pT_t[:, 2 * pair:2 * pair + 2, :], sm[:], AF.Exp
                            )
                        else:
                            nc.scalar.activation(
                                pT_t[:, 2 * pair:2 * pair + 2, :], pss[:], AF.Exp,
                                scale=0.125,
                            )
                    # PV tracking sweep for subs 0,1 (half 0)
                    for sub in range(2):
                        for hd in range(4):
                            nc.tensor.matmul(
                                oraw0[:, sub * 4 + hd, 0:65],
                                lhsT=pT_t[:, hd, sub * 128:(sub + 1) * 128],
                                rhs=v_sb[:, jc, 0:65],
                                start=(jc == 0), stop=(jc == 15),
                            )
                    if ic < 3 and jc % 4 == 3:
                        q_chain(4 * (ic + 1) + jc // 4)
                finish_half(ic, 0, oraw0)
                # PV bulk sweep for subs 2,3 (half 1)
                oraw1 = ps.tile([128, 8, 128], F32, tag="oraw")
                for jc in range(16):
                    for sub in range(2, 4):
                        for hd in range(4):
                            nc.tensor.matmul(
                                oraw1[:, (sub - 2) * 4 + hd, 0:65],
                                lhsT=pts[jc][:, hd, sub * 128:(sub + 1) * 128],
                                rhs=v_sb[:, jc, 0:65],
                                start=(jc == 0), stop=(jc == 15),
                            )
                finish_half(ic, 1, oraw1)

    nc.compile()
    return nc


def _get(use_mask: bool, use_bias: bool = False):
    key = (use_mask, use_bias)
    if key not in _cache:
        _cache[key] = _build(use_mask, use_bias)
    return _cache[key]


def _host_prep(hidden_state, attention_mask, Wq, bq, Wk, bk, Wv, bv, Wo,
               use_mask, use_bias):
    """Build the 8 per-core input maps."""
    half_q, half_k = HID // 2, (HKV * D) // 2  # 512, 128
    inv_q = ROPE_BASE ** (-np.arange(half_q, dtype=np.float64) / half_q)
    inv_k = ROPE_BASE ** (-np.arange(half_k, dtype=np.float64) / half_k)
    s_idx = np.arange(S, dtype=np.float64)
    ang_q = inv_q[:, None] * s_idx[None, :]  # [512, S] freq-major
    ang_k = inv_k[:, None] * s_idx[None, :]  # [128, S]
    cos_q, sin_q = np.cos(ang_q), np.sin(ang_q)
    cos_k, sin_k = np.cos(ang_k), np.sin(ang_k)

    in_maps = []
    for core in range(8):
        b, j = core // 4, core % 4
        own_q = np.arange(j * 256, (j + 1) * 256)
        par_q = own_q + 512 if j < 2 else own_q - 512
        fidx_q = own_q if j < 2 else own_q - 512
        sign = -1.0 if j < 2 else 1.0
        own_k = np.arange(j * 64, (j + 1) * 64)
        par_k = own_k + 128 if j < 2 else own_k - 128
        fidx_k = own_k if j < 2 else own_k - 128

        hTc = np.ascontiguousarray(hidden_state[b].T).astype(NB).reshape(8, 128, S)
        wq_c = np.concatenate([Wq[:, own_q], Wq[:, par_q]], axis=1)
        wq_c = wq_c.astype(NB).reshape(8, 128, QSEL)
        wk_c = np.concatenate([Wk[:, own_k], Wk[:, par_k]], axis=1)
        wk_c = wk_c.astype(NB).reshape(8, 128, 128)
        wv_c = Wv[:, own_k].astype(NB).reshape(8, 128, 64)
        wo_c = Wo[j * 256:(j + 1) * 256, :].astype(NB).reshape(2, 128, HID)
        # natural-layout cos/sin: [sc, seq128, {cos, signed sin}, feat]
        csq_c = np.stack(
            [cos_q[fidx_q].T, (sign * sin_q[fidx_q]).T], axis=1
        )  # [S, 2, 256]
        csq_c = csq_c.astype(NB).reshape(NSC, 128, 2, 256)
        csk_c = np.stack(
            [cos_k[fidx_k].T, (sign * sin_k[fidx_k]).T], axis=1
        )  # [S, 2, 64]
        csk_c = csk_c.astype(NB).reshape(NSC, 128, 2, 64)

        m = {
            "hT": hTc, "wq": wq_c, "wk": wk_c, "wv": wv_c, "wo": wo_c,
            "csq": csq_c, "csk": csk_c,
        }
        if use_mask:
            mT = np.ascontiguousarray(attention_mask[b].T).astype(np.float32)
            m["mk"] = mT.reshape(NSC, 128, S)
        if use_bias:
            m["brq"] = np.concatenate([bq[own_q], bq[par_q]]).astype(NB).reshape(1, QSEL)
            m["brk"] = np.concatenate([bk[own_k], bk[par_k]]).astype(NB).reshape(1, 128)
            m["brv"] = bv[own_k].astype(NB).reshape(1, 64)
        in_maps.append(m)
    return in_maps


def kernel(hidden_state, attention_mask, Wq, bq, Wk, bk, Wv, bv, Wo, bo):
    from concourse.bass_utils import run_bass_kernel_spmd

    hidden_state = np.asarray(hidden_state, dtype=np.float32)
    attention_mask = np.asarray(attention_mask, dtype=np.float32)
    Wq, bq = np.asarray(Wq, np.float32), np.asarray(bq, np.float32)
    Wk, bk = np.asarray(Wk, np.float32), np.asarray(bk, np.float32)
    Wv, bv = np.asarray(Wv, np.float32), np.asarray(bv, np.float32)
    Wo, bo = np.asarray(Wo, np.float32), np.asarray(bo, np.float32)
    use_mask = bool(np.any(attention_mask))
    use_bias = bool(np.any(bq) or np.any(bk) or np.any(bv))
    nc = _get(use_mask, use_bias)
    in_maps = _host_prep(
        hidden_state, attention_mask, Wq, bq, Wk, bk, Wv, bv, Wo,
        use_mask, use_bias,
    )
    res = run_bass_kernel_spmd(nc, in_maps, list(range(8)))
    out = np.zeros((B, S, HID), dtype=np.float32)
    for core in range(8):
        out[core // 4] += res.results[core]["y"].reshape(S, HID)
    out += bo[None, None, :]
    return out


# revision 6
# speedup vs baseline: 1.4352x; 1.0239x over previous
"""GQA attention kernel for Trainium2, 8 NeuronCores.

Sharding: data-parallel over batch (B=2) x tensor-parallel over KV heads
(HKV=4) -> 8 cores.  Core c handles batch b=c//4, kv-head j=c%4 with its
G=4 query heads.  out_proj is row-parallel; partials are reduced on host.

Layout strategy (v2):
  - Projections in NATURAL orientation (out[seq, feat]): lhsT = hiddenT
    chunk, rhs = W chunk.  RoPE and rmsnorm then operate along the free
    dim (cheap DVE/Pool ops, no partition reductions).
  - rsqrt for rmsnorm is exp(-0.5*ln(x)) on ACT: both funcs live in the
    natural_log_exp_and_others activation table together with the softmax
    Exp, so the ACT engine never reloads its table.
  - qT / kT for the scores matmuls are produced by DMA-transpose
    (crossbar) instructions; kT's row-64..127 duplicate is folded into
    the same transpose by duplicating kn columns beforehand.
  - scoresT[key, q] = kT^T @ qT per head, exp on ACT (the hard floor:
    ~110us of exp at 0.833 ns/elem), probabilities pT kept in SBUF for a
    full 512-q block.
  - PV in flipped orientation: out[q, d+1] with lhsT = pT chunk,
    rhs = v (with ones column -> denominator lands as column 64).  N=65
    per matmul instead of 512 -> half the PE rows of the baseline.
    Normalization is a per-partition reciprocal + broadcast multiply.
  - oT via DMA-transpose feeds a row-parallel out_proj; partials DMA'd
    per 128-row chunk.
PSUM budget (8 banks): pq 1 | scoresA 2 | scoresB 2 | oraw 2 | y 1.
The lead-in k/v/q chains round-robin across all five slots.
"""

import numpy as np
import ml_dtypes

import concourse.bacc as bacc
import concourse.mybir as mybir
from concourse.tile import TileContext

BF16 = mybir.dt.bfloat16
F32 = mybir.dt.float32
AL = mybir.AluOpType
AF = mybir.ActivationFunctionType
AX = mybir.AxisListType

B, S, HID = 2, 2048, 1024
H, HKV, D = 16, 4, 64
G = H // HKV          # 4 query heads per kv head
QSEL = 2 * G * D      # 512: own 256 cols + rope-partner 256 cols
ROPE_BASE = 10000.0
EPS = float(np.finfo(np.float32).eps)
NSC = S // 128        # 16 seq chunks
NIC = 4               # 512-wide q blocks

NB = ml_dtypes.bfloat16

_cache: dict = {}


def _build(use_mask: bool, use_bias: bool):
    nc = bacc.Bacc("TRN2", target_bir_lowering=False)

    hT = nc.dram_tensor("hT", [8, 128, S], BF16, kind="ExternalInput")
    wq = nc.dram_tensor("wq", [8, 128, QSEL], BF16, kind="ExternalInput")
    wk = nc.dram_tensor("wk", [8, 128, 128], BF16, kind="ExternalInput")
    wv = nc.dram_tensor("wv", [8, 128, 64], BF16, kind="ExternalInput")
    wo = nc.dram_tensor("wo", [2, 128, HID], BF16, kind="ExternalInput")
    csq = nc.dram_tensor("csq", [NSC, 128, 2, 256], BF16, kind="ExternalInput")
    csk = nc.dram_tensor("csk", [NSC, 128, 2, 64], BF16, kind="ExternalInput")
    y = nc.dram_tensor("y", [NSC, 128, HID], F32, kind="ExternalOutput")
    mk = (
        nc.dram_tensor("mk", [NSC, 128, S], F32, kind="ExternalInput")
        if use_mask
        else None
    )
    if use_bias:
        brq = nc.dram_tensor("brq", [1, QSEL], BF16, kind="ExternalInput")
        brk = nc.dram_tensor("brk", [1, 128], BF16, kind="ExternalInput")
        brv = nc.dram_tensor("brv", [1, 64], BF16, kind="ExternalInput")

    with TileContext(nc) as tc:
        with (
            tc.tile_pool(name="const", bufs=1) as cp,
            tc.tile_pool(name="proj", bufs=1) as pj,
            tc.tile_pool(name="rt", bufs=3) as rt,
            tc.tile_pool(name="ro", bufs=6) as rop,
            tc.tile_pool(name="stat", bufs=3) as stp,
            tc.tile_pool(name="pT", bufs=20) as ptp,
            tc.tile_pool(name="onat", bufs=3) as onp_,
            tc.tile_pool(name="oTp", bufs=3) as otp,
            tc.tile_pool(name="ysb", bufs=2) as yp,
            tc.tile_pool(name="maskp", bufs=3) as mp,
            tc.tile_pool(name="ps", bufs=1, space="PSUM") as ps,
        ):
            # ---- persistent tiles ------------------------------------
            wo_sb = cp.tile([128, 2, HID], BF16)
            for cc in range(2):
                nc.sync.dma_start(out=wo_sb[:, cc, :], in_=wo[cc])
            v_sb = cp.tile([128, NSC, 66], BF16)
            nc.vector.memset(v_sb[:, :, 64:65], 1.0)
            eps_sb = cp.tile([128, 1], F32)
            nc.vector.memset(eps_sb[:], EPS)
            qT = cp.tile([128, 2, S], BF16)
            kT = cp.tile([128, S], BF16)

            # ---- projection-phase constants --------------------------
            hT_sb = pj.tile([128, 8, S], BF16)
            for ko in range(8):
                nc.sync.dma_start(out=hT_sb[:, ko, :], in_=hT[ko])
            wk_sb = pj.tile([128, 8, 128], BF16)
            nc.sync.dma_start(out=wk_sb[:], in_=wk[:].rearrange("a b c -> b a c"))
            wv_sb = pj.tile([128, 8, 64], BF16)
            nc.sync.dma_start(out=wv_sb[:], in_=wv[:].rearrange("a b c -> b a c"))
            csk_sb = pj.tile([128, NSC, 2, 64], BF16)
            nc.sync.dma_start(out=csk_sb[:], in_=csk[:].rearrange("a b c d -> b a c d"))
            wq_sb = pj.tile([128, 8, QSEL], BF16)
            nc.sync.dma_start(out=wq_sb[:], in_=wq[:].rearrange("a b c -> b a c"))
            csq_sb = pj.tile([128, NSC, 2, 256], BF16)
            nc.sync.dma_start(out=csq_sb[:], in_=csq[:].rearrange("a b c d -> b a c d"))
            if use_bias:
                ones1 = cp.tile([1, 128], BF16)
                nc.vector.memset(ones1[:], 1.0)
                brq_sb = cp.tile([1, QSEL], BF16)
                nc.sync.dma_start(out=brq_sb[:], in_=brq[:])
                brk_sb = cp.tile([1, 128], BF16)
                nc.sync.dma_start(out=brk_sb[:], in_=brk[:])
                brv_sb = cp.tile([1, 64], BF16)
                nc.sync.dma_start(out=brv_sb[:], in_=brv[:])

            # PSUM slot round-robin for the lead-in projection chains
            SLOTS = ["scA", "scB", "oraw", "py", "pq"]
            slot_i = [0]

            def next_slot():
                s = SLOTS[slot_i[0] % len(SLOTS)]
                slot_i[0] += 1
                return s

            def rsqrt_batch(rm, tag):
                """rm: [128, n] f32 sums of squares/64 -> (rm+eps)^-0.5 via
                exp(-0.5 * ln(rm + eps)); stays in the exp table set."""
                n = rm.shape[1]
                ln_t = stp.tile([128, n], F32, tag=tag + "_ln")
                nc.scalar.activation(ln_t[:], rm[:], AF.Ln, bias=eps_sb[:])
                rc = stp.tile([128, n], F32, tag=tag + "_rc")
                nc.scalar.activation(rc[:], ln_t[:], AF.Exp, scale=-0.5)
                return rc

            def kv_sub(sc, rmk4, i, kros):
                ssl = slice(sc * 128, (sc + 1) * 128)
                pk = ps.tile([128, 2, 64], F32, tag=next_slot())
                pv = ps.tile([128, 64], F32, tag=next_slot())
                for ko in range(8):
                    st, sp = ko == 0, (ko == 7 and not use_bias)
                    nc.tensor.matmul(
                        pk[:], lhsT=hT_sb[:, ko, ssl], rhs=wk_sb[:, ko, :],
                        start=st, stop=sp,
                    )
                for ko in range(8):
                    st, sp = ko == 0, (ko == 7 and not use_bias)
                    nc.tensor.matmul(
                        pv[:], lhsT=hT_sb[:, ko, ssl], rhs=wv_sb[:, ko, :],
                        start=st, stop=sp,
                    )
                if use_bias:
                    nc.tensor.matmul(pk[:], lhsT=ones1[:], rhs=brk_sb[:],
                                     start=False, stop=True)
                    nc.tensor.matmul(pv[:], lhsT=ones1[:], rhs=brv_sb[:],
                                     start=False, stop=True)
                t12k = rt.tile([128, 2, 64], BF16, tag="t12k")
                nc.vector.tensor_tensor(t12k[:], pk[:], csk_sb[:, sc, :, :], AL.mult)
                kro = rop.tile([128, 64], BF16, tag="kro")
                nc.gpsimd.tensor_tensor(kro[:], t12k[:, 0, :], t12k[:, 1, :], AL.add)
                kros.append(kro)
                sqk = rt.tile([128, 64], BF16, tag="sqk")
                nc.gpsimd.scalar_tensor_tensor(
                    sqk[:], kro[:], 1.0 / 64.0, kro[:], AL.mult, AL.mult
                )
                nc.vector.tensor_reduce(rmk4[:, i:i + 1], sqk[:], AX.X, AL.add)
                nc.vector.tensor_copy(v_sb[:, sc, 0:64], pv[:])

            def kv_fin(g, rmk4, kros):
                rck = rsqrt_batch(rmk4, "rck")
                for i in range(4):
                    sc = 4 * g + i
                    kn2 = rt.tile([128, 2, 64], BF16, tag="kn2")
                    nc.vector.tensor_scalar_mul(kn2[:, 0, :], kros[i][:], rck[:, i:i + 1])
                    nc.gpsimd.tensor_copy(kn2[:, 1, :], kn2[:, 0, :])
                    nc.sync.dma_start_transpose(
                        out=kT[:, sc * 128:(sc + 1) * 128], in_=kn2[:]
                    )

            def q_sub(sc, rms16, i, qros, lead=False):
                ssl = slice(sc * 128, (sc + 1) * 128)
                pq = ps.tile([128, 2, 256], F32, tag=(next_slot() if lead else "pq"))
                for ko in range(8):
                    st, sp = ko == 0, (ko == 7 and not use_bias)
                    nc.tensor.matmul(
                        pq[:], lhsT=hT_sb[:, ko, ssl], rhs=wq_sb[:, ko, :],
                        start=st, stop=sp,
                    )
                if use_bias:
                    nc.tensor.matmul(pq[:], lhsT=ones1[:], rhs=brq_sb[:],
                                     start=False, stop=True)
                t12 = rt.tile([128, 2, 256], BF16, tag="t12")
                nc.vector.tensor_tensor(t12[:], pq[:], csq_sb[:, sc, :, :], AL.mult)
                qro = rop.tile([128, 4, 64], BF16, tag="qro")
                nc.gpsimd.tensor_tensor(
                    qro[:].rearrange("p h d -> p (h d)"), t12[:, 0, :], t12[:, 1, :],
                    AL.add,
                )
                qros.append(qro)
                sqq = rt.tile([128, 4, 64], BF16, tag="sqq")
                nc.gpsimd.scalar_tensor_tensor(
                    sqq[:], qro[:], 1.0 / 64.0, qro[:], AL.mult, AL.mult
                )
                nc.vector.tensor_reduce(rms16[:, 4 * i:4 * i + 4], sqq[:], AX.X, AL.add)

            def q_fin(ic, rms16, qros):
                rcq = rsqrt_batch(rms16, "rcq")
                for i in range(4):
                    sc = 4 * ic + i
                    qn = rt.tile([128, 4, 64], BF16, tag="qn")
                    nc.vector.tensor_tensor(
                        qn[:], qros[i][:],
                        rcq[:, 4 * i:4 * i + 4, None].to_broadcast((128, 4, 64)),
                        AL.mult,
                    )
                    nc.sync.dma_start_transpose(
                        out=qT[:, :, sc * 128:(sc + 1) * 128], in_=qn[:]
                    )

            def finish_half(ic, h, oraw):
                # oraw: [128, 8, 128] psum, slices (s2, hd) at s2*4+hd, col 64 = denom
                rcp = stp.tile([128, 8], F32, tag="rcp")
                nc.vector.reciprocal(rcp[:], oraw[:, :, 64:65])
                for s2 in range(2):
                    sub = 2 * h + s2
                    onat = onp_.tile([128, 4, 64], BF16, tag="onat")
                    nc.vector.tensor_tensor(
                        onat[:], oraw[:, s2 * 4:(s2 + 1) * 4, 0:64],
                        rcp[:, s2 * 4:(s2 + 1) * 4, None].to_broadcast((128, 4, 64)),
                        AL.mult,
                    )
                    oTt = otp.tile([128, 2, 128], BF16, tag="oTt")
                    nc.sync.dma_start_transpose(out=oTt[:], in_=onat[:])
                    ysb = yp.tile([128, HID], F32, tag="ysb")
                    for ec in range(2):
                        py = ps.tile([128, 512], F32, tag="py")
                        for cc in range(2):
                            nc.tensor.matmul(
                                py[:], lhsT=oTt[:, cc, :],
                                rhs=wo_sb[:, cc, ec * 512:(ec + 1) * 512],
                                start=(cc == 0), stop=(cc == 1),
                            )
                        nc.vector.tensor_copy(ysb[:, ec * 512:(ec + 1) * 512], py[:])
                    nc.sync.dma_start(out=y[ic * 4 + sub], in_=ysb[:])

            # ---- lead-in: k/v for all chunks, then q for ic 0 --------
            for g in range(4):
                rmk4 = stp.tile([128, 4], F32, tag="rmk4")
                kros = []
                for i in range(4):
                    kv_sub(4 * g + i, rmk4, i, kros)
                kv_fin(g, rmk4, kros)
            rms16 = stp.tile([128, 16], F32, tag="rms16")
            qros = []
            for i in range(4):
                q_sub(i, rms16, i, qros, lead=True)
            q_fin(0, rms16, qros)

            # ---- attention -------------------------------------------
            for ic in range(NIC):
                isl = slice(ic * 512, (ic + 1) * 512)
                pts = []
                oraw0 = ps.tile([128, 8, 128], F32, tag="oraw")
                if ic < 3:
                    nrms = stp.tile([128, 16], F32, tag="rms16")
                    nqros = []
                for jc in range(16):
                    pT_t = ptp.tile([128, 4, 512], BF16, tag="pT")
                    pts.append(pT_t)
                    if use_mask:
                        mkt = mp.tile([128, 512], F32, tag="mkt")
                        nc.sync.dma_start(out=mkt[:], in_=mk[jc][:, isl])
                    for pair in range(2):
                        pss = ps.tile([128, 2, 512], F32,
                                      tag=("scA" if pair == 0 else "scB"))
                        for hh in range(2):
                            rows = slice(64 * hh, 64 * hh + 64)
                            nc.tensor.matmul(
                                pss[:, hh, :],
                                lhsT=kT[rows, jc * 128:(jc + 1) * 128],
                                rhs=qT[rows, pair, isl],
                                start=True, stop=True,
                            )
                        if use_mask:
                            sm = mp.tile([128, 2, 512], F32, tag="sm")
                            nc.vector.scalar_tensor_tensor(
                                sm[:], pss[:], 0.125,
                                mkt[:, None, :].to_broadcast((128, 2, 512)),
                                AL.mult, AL.add,
                            )
                            nc.scalar.activation(
                                pT_t[:, 2 * pair:2 * pair + 2, :], sm[:], AF.Exp
                            )
                        else:
                            nc.scalar.activation(
                                pT_t[:, 2 * pair:2 * pair + 2, :], pss[:], AF.Exp,
                                scale=0.125,
                            )
                    # PV tracking sweep for subs 0,1 (half 0)
                    for sub in range(2):
                        for hd in range(4):
                            nc.tensor.matmul(
                                oraw0[:, sub * 4 + hd, 0:65],
                                lhsT=pT_t[:, hd, sub * 128:(sub + 1) * 128],
                                rhs=v_sb[:, jc, 0:65],
                                start=(jc == 0), stop=(jc == 15),
                            )
                    # interleave next block's q chains early in the sweep
                    if ic < 3:
                        if jc in (1, 3, 5, 7):
                            q_sub(4 * (ic + 1) + jc // 2, nrms, jc // 2, nqros)
                        elif jc == 9:
                            q_fin(ic + 1, nrms, nqros)
                finish_half(ic, 0, oraw0)
                # PV bulk sweep for subs 2,3 (half 1)
                oraw1 = ps.tile([128, 8, 128], F32, tag="oraw")
                for jc in range(16):
                    for sub in range(2, 4):
                        for hd in range(4):
                            nc.tensor.matmul(
                                oraw1[:, (sub - 2) * 4 + hd, 0:65],
                                lhsT=pts[jc][:, hd, sub * 128:(sub + 1) * 128],
                                rhs=v_sb[:, jc, 0:65],
                                start=(jc == 0), stop=(jc == 15),
                            )
                finish_half(ic, 1, oraw1)

    nc.compile()
    return nc


def _get(use_mask: bool, use_bias: bool = False):
    key = (use_mask, use_bias)
    if key not in _cache:
        _cache[key] = _build(use_mask, use_bias)
    return _cache[key]


def _host_prep(hidden_state, attention_mask, Wq, bq, Wk, bk, Wv, bv, Wo,
               use_mask, use_bias):
    """Build the 8 per-core input maps."""
    half_q, half_k = HID // 2, (HKV * D) // 2  # 512, 128
    inv_q = ROPE_BASE ** (-np.arange(half_q, dtype=np.float64) / half_q)
    inv_k = ROPE_BASE ** (-np.arange(half_k, dtype=np.float64) / half_k)
    s_idx = np.arange(S, dtype=np.float64)
    ang_q = inv_q[:, None] * s_idx[None, :]  # [512, S] freq-major
    ang_k = inv_k[:, None] * s_idx[None, :]  # [128, S]
    cos_q, sin_q = np.cos(ang_q), np.sin(ang_q)
    cos_k, sin_k = np.cos(ang_k), np.sin(ang_k)

    in_maps = []
    for core in range(8):
        b, j = core // 4, core % 4
        own_q = np.arange(j * 256, (j + 1) * 256)
        par_q = own_q + 512 if j < 2 else own_q - 512
        fidx_q = own_q if j < 2 else own_q - 512
        sign = -1.0 if j < 2 else 1.0
        own_k = np.arange(j * 64, (j + 1) * 64)
        par_k = own_k + 128 if j < 2 else own_k - 128
        fidx_k = own_k if j < 2 else own_k - 128

        hTc = np.ascontiguousarray(hidden_state[b].T).astype(NB).reshape(8, 128, S)
        wq_c = np.concatenate([Wq[:, own_q], Wq[:, par_q]], axis=1)
        wq_c = wq_c.astype(NB).reshape(8, 128, QSEL)
        wk_c = np.concatenate([Wk[:, own_k], Wk[:, par_k]], axis=1)
        wk_c = wk_c.astype(NB).reshape(8, 128, 128)
        wv_c = Wv[:, own_k].astype(NB).reshape(8, 128, 64)
        wo_c = Wo[j * 256:(j + 1) * 256, :].astype(NB).reshape(2, 128, HID)
        # natural-layout cos/sin: [sc, seq128, {cos, signed sin}, feat]
        csq_c = np.stack(
            [cos_q[fidx_q].T, (sign * sin_q[fidx_q]).T], axis=1
        )  # [S, 2, 256]
        csq_c = csq_c.astype(NB).reshape(NSC, 128, 2, 256)
        csk_c = np.stack(
            [cos_k[fidx_k].T, (sign * sin_k[fidx_k]).T], axis=1
        )  # [S, 2, 64]
        csk_c = csk_c.astype(NB).reshape(NSC, 128, 2, 64)

        m = {
            "hT": hTc, "wq": wq_c, "wk": wk_c, "wv": wv_c, "wo": wo_c,
            "csq": csq_c, "csk": csk_c,
        }
        if use_mask:
            mT = np.ascontiguousarray(attention_mask[b].T).astype(np.float32)
            m["mk"] = mT.reshape(NSC, 128, S)
        if use_bias:
            m["brq"] = np.concatenate([bq[own_q], bq[par_q]]).astype(NB).reshape(1, QSEL)
            m["brk"] = np.concatenate([bk[own_k], bk[par_k]]).astype(NB).reshape(1, 128)
            m["brv"] = bv[own_k].astype(NB).reshape(1, 64)
        in_maps.append(m)
    return in_maps


def kernel(hidden_state, attention_mask, Wq, bq, Wk, bk, Wv, bv, Wo, bo):
    from concourse.bass_utils import run_bass_kernel_spmd

    hidden_state = np.asarray(hidden_state, dtype=np.float32)
    attention_mask = np.asarray(attention_mask, dtype=np.float32)
    Wq, bq = np.asarray(Wq, np.float32), np.asarray(bq, np.float32)
    Wk, bk = np.asarray(Wk, np.float32), np.asarray(bk, np.float32)
    Wv, bv = np.asarray(Wv, np.float32), np.asarray(bv, np.float32)
    Wo, bo = np.asarray(Wo, np.float32), np.asarray(bo, np.float32)
    use_mask = bool(np.any(attention_mask))
    use_bias = bool(np.any(bq) or np.any(bk) or np.any(bv))
    nc = _get(use_mask, use_bias)
    in_maps = _host_prep(
        hidden_state, attention_mask, Wq, bq, Wk, bk, Wv, bv, Wo,
        use_mask, use_bias,
    )
    res = run_bass_kernel_spmd(nc, in_maps, list(range(8)))
    out = np.zeros((B, S, HID), dtype=np.float32)
    for core in range(8):
        out[core // 4] += res.results[core]["y"].reshape(S, HID)
    out += bo[None, None, :]
    return out


# revision 13
# speedup vs baseline: 1.5108x; 1.0527x over previous
"""GQA attention kernel for Trainium2, 8 NeuronCores.

Sharding: data-parallel over batch (B=2) x tensor-parallel over KV heads
(HKV=4) -> 8 cores.  Core c handles batch b=c//4, kv-head j=c%4 with its
G=4 query heads.  out_proj is row-parallel; partials are reduced on host.

Layout strategy (v2):
  - Projections in NATURAL orientation (out[seq, feat]): lhsT = hiddenT
    chunk, rhs = W chunk.  RoPE and rmsnorm then operate along the free
    dim (cheap DVE/Pool ops, no partition reductions).
  - rsqrt for rmsnorm is exp(-0.5*ln(x)) on ACT: both funcs live in the
    natural_log_exp_and_others activation table together with the softmax
    Exp, so the ACT engine never reloads its table.
  - qT / kT for the scores matmuls are produced by DMA-transpose
    (crossbar) instructions; kT's row-64..127 duplicate is folded into
    the same transpose by duplicating kn columns beforehand.
  - scoresT[key, q] = kT^T @ qT per head, exp on ACT (the hard floor:
    ~110us of exp at 0.833 ns/elem), probabilities pT kept in SBUF for a
    full 512-q block.
  - PV in flipped orientation: out[q, d+1] with lhsT = pT chunk,
    rhs = v (with ones column -> denominator lands as column 64).  N=65
    per matmul instead of 512 -> half the PE rows of the baseline.
    Normalization is a per-partition reciprocal + broadcast multiply.
  - oT via DMA-transpose feeds a row-parallel out_proj; partials DMA'd
    per 128-row chunk.
PSUM budget (8 banks): pq 1 | scoresA 2 | scoresB 2 | oraw 2 | y 1.
The lead-in k/v/q chains round-robin across all five slots.
"""

import numpy as np
import ml_dtypes

import concourse.bacc as bacc
import concourse.mybir as mybir
from concourse.tile import TileContext

BF16 = mybir.dt.bfloat16
F32 = mybir.dt.float32
AL = mybir.AluOpType
AF = mybir.ActivationFunctionType
AX = mybir.AxisListType

B, S, HID = 2, 2048, 1024
H, HKV, D = 16, 4, 64
G = H // HKV          # 4 query heads per kv head
QSEL = 2 * G * D      # 512: own 256 cols + rope-partner 256 cols
ROPE_BASE = 10000.0
EPS = float(np.finfo(np.float32).eps)
NSC = S // 128        # 16 seq chunks
NIC = 4               # 512-wide q blocks

NB = ml_dtypes.bfloat16

_cache: dict = {}


def _build(use_mask: bool, use_bias: bool):
    nc = bacc.Bacc("TRN2", target_bir_lowering=False)

    hT = nc.dram_tensor("hT", [8, 128, S], BF16, kind="ExternalInput")
    wq = nc.dram_tensor("wq", [8, 128, QSEL], BF16, kind="ExternalInput")
    wk = nc.dram_tensor("wk", [8, 128, 128], BF16, kind="ExternalInput")
    wv = nc.dram_tensor("wv", [8, 128, 64], BF16, kind="ExternalInput")
    wo = nc.dram_tensor("wo", [2, 128, HID], BF16, kind="ExternalInput")
    csq = nc.dram_tensor("csq", [NSC, 128, 2, 256], BF16, kind="ExternalInput")
    csk = nc.dram_tensor("csk", [NSC, 128, 2, 64], BF16, kind="ExternalInput")
    y = nc.dram_tensor("y", [NSC, 128, HID], F32, kind="ExternalOutput")
    mk = (
        nc.dram_tensor("mk", [NSC, 128, S], F32, kind="ExternalInput")
        if use_mask
        else None
    )
    if use_bias:
        brq = nc.dram_tensor("brq", [1, QSEL], BF16, kind="ExternalInput")
        brk = nc.dram_tensor("brk", [1, 128], BF16, kind="ExternalInput")
        brv = nc.dram_tensor("brv", [1, 64], BF16, kind="ExternalInput")

    with TileContext(nc) as tc:
        with (
            tc.tile_pool(name="const", bufs=1) as cp,
            tc.tile_pool(name="proj", bufs=1) as pj,
            tc.tile_pool(name="rt", bufs=3) as rt,
            tc.tile_pool(name="ro", bufs=6) as rop,
            tc.tile_pool(name="stat", bufs=3) as stp,
            tc.tile_pool(name="pT", bufs=20) as ptp,
            tc.tile_pool(name="onat", bufs=3) as onp_,
            tc.tile_pool(name="oTp", bufs=3) as otp,
            tc.tile_pool(name="ysb", bufs=2) as yp,
            tc.tile_pool(name="maskp", bufs=3) as mp,
            tc.tile_pool(name="ps", bufs=1, space="PSUM") as ps,
        ):
            # ---- persistent tiles ------------------------------------
            wo_sb = cp.tile([128, 2, HID], BF16)
            for cc in range(2):
                nc.sync.dma_start(out=wo_sb[:, cc, :], in_=wo[cc])
            v_sb = cp.tile([128, NSC, 66], BF16)
            nc.vector.memset(v_sb[:, :, 64:65], 1.0)
            eps_sb = cp.tile([128, 1], F32)
            nc.vector.memset(eps_sb[:], EPS)
            qT = cp.tile([128, 2, S], BF16)
            kT = cp.tile([128, S], BF16)

            # ---- projection-phase constants --------------------------
            # small weights first so the first k/v matmuls start early
            wk_sb = pj.tile([128, 8, 128], BF16)
            nc.sync.dma_start(out=wk_sb[:], in_=wk[:].rearrange("a b c -> b a c"))
            wv_sb = pj.tile([128, 8, 64], BF16)
            nc.sync.dma_start(out=wv_sb[:], in_=wv[:].rearrange("a b c -> b a c"))
            csk_sb = pj.tile([128, NSC, 2, 64], BF16)
            nc.sync.dma_start(out=csk_sb[:], in_=csk[:].rearrange("a b c d -> b a c d"))
            hT_sb = pj.tile([128, 8, S], BF16)
            for ko in range(8):
                nc.sync.dma_start(out=hT_sb[:, ko, :], in_=hT[ko])
            wq_sb = pj.tile([128, 8, QSEL], BF16)
            nc.sync.dma_start(out=wq_sb[:], in_=wq[:].rearrange("a b c -> b a c"))
            csq_sb = pj.tile([128, NSC, 2, 256], BF16)
            nc.sync.dma_start(out=csq_sb[:], in_=csq[:].rearrange("a b c d -> b a c d"))
            if use_bias:
                ones1 = cp.tile([1, 128], BF16)
                nc.vector.memset(ones1[:], 1.0)
                brq_sb = cp.tile([1, QSEL], BF16)
                nc.sync.dma_start(out=brq_sb[:], in_=brq[:])
                brk_sb = cp.tile([1, 128], BF16)
                nc.sync.dma_start(out=brk_sb[:], in_=brk[:])
                brv_sb = cp.tile([1, 64], BF16)
                nc.sync.dma_start(out=brv_sb[:], in_=brv[:])

            # PSUM slot round-robin for the lead-in projection chains
            SLOTS = ["scA", "scB", "oraw", "py", "pq"]
            slot_i = [0]

            def next_slot():
                s = SLOTS[slot_i[0] % len(SLOTS)]
                slot_i[0] += 1
                return s

            def rsqrt_batch(rm, tag):
                """rm: [128, n] f32 sums of squares/64 -> (rm+eps)^-0.5 via
                exp(-0.5 * ln(rm + eps)); stays in the exp table set."""
                n = rm.shape[1]
                ln_t = stp.tile([128, n], F32, tag=tag + "_ln")
                nc.scalar.activation(ln_t[:], rm[:], AF.Ln, bias=eps_sb[:])
                rc = stp.tile([128, n], F32, tag=tag + "_rc")
                nc.scalar.activation(rc[:], ln_t[:], AF.Exp, scale=-0.5)
                return rc

            def kv_sub(sc, rmk4, i, kros):
                ssl = slice(sc * 128, (sc + 1) * 128)
                pk = ps.tile([128, 2, 64], F32, tag=next_slot())
                pv = ps.tile([128, 64], F32, tag=next_slot())
                for ko in range(8):
                    st, sp = ko == 0, (ko == 7 and not use_bias)
                    nc.tensor.matmul(
                        pk[:], lhsT=hT_sb[:, ko, ssl], rhs=wk_sb[:, ko, :],
                        start=st, stop=sp,
                    )
                for ko in range(8):
                    st, sp = ko == 0, (ko == 7 and not use_bias)
                    nc.tensor.matmul(
                        pv[:], lhsT=hT_sb[:, ko, ssl], rhs=wv_sb[:, ko, :],
                        start=st, stop=sp,
                    )
                if use_bias:
                    nc.tensor.matmul(pk[:], lhsT=ones1[:], rhs=brk_sb[:],
                                     start=False, stop=True)
                    nc.tensor.matmul(pv[:], lhsT=ones1[:], rhs=brv_sb[:],
                                     start=False, stop=True)
                t12k = rt.tile([128, 2, 64], BF16, tag="t12k")
                nc.vector.tensor_tensor(t12k[:], pk[:], csk_sb[:, sc, :, :], AL.mult)
                kro = rop.tile([128, 64], BF16, tag="kro")
                nc.gpsimd.tensor_tensor(kro[:], t12k[:, 0, :], t12k[:, 1, :], AL.add)
                kros.append(kro)
                sqk = rt.tile([128, 64], BF16, tag="sqk")
                nc.gpsimd.scalar_tensor_tensor(
                    sqk[:], kro[:], 1.0 / 64.0, kro[:], AL.mult, AL.mult
                )
                nc.vector.tensor_reduce(rmk4[:, i:i + 1], sqk[:], AX.X, AL.add)
                nc.vector.tensor_copy(v_sb[:, sc, 0:64], pv[:])

            def kv_fin(g, rmk4, kros):
                rck = rsqrt_batch(rmk4, "rck")
                for i in range(4):
                    sc = 4 * g + i
                    kn2 = rt.tile([128, 2, 64], BF16, tag="kn2")
                    nc.vector.tensor_scalar_mul(kn2[:, 0, :], kros[i][:], rck[:, i:i + 1])
                    nc.gpsimd.tensor_copy(kn2[:, 1, :], kn2[:, 0, :])
                    nc.sync.dma_start_transpose(
                        out=kT[:, sc * 128:(sc + 1) * 128], in_=kn2[:]
                    )

            def q_sub(sc, rms16, i, qros, lead=False):
                ssl = slice(sc * 128, (sc + 1) * 128)
                pq = ps.tile([128, 2, 256], F32, tag=(next_slot() if lead else "pq"))
                for ko in range(8):
                    st, sp = ko == 0, (ko == 7 and not use_bias)
                    nc.tensor.matmul(
                        pq[:], lhsT=hT_sb[:, ko, ssl], rhs=wq_sb[:, ko, :],
                        start=st, stop=sp,
                    )
                if use_bias:
                    nc.tensor.matmul(pq[:], lhsT=ones1[:], rhs=brq_sb[:],
                                     start=False, stop=True)
                q_sub_tail(sc, rms16, i, qros, pq)

            def q_sub_mm(sc, pq_box, ko0, lead=False):
                """two accumulation matmuls of the q projection for chunk sc"""
                ssl = slice(sc * 128, (sc + 1) * 128)
                if ko0 == 0:
                    pq_box.append(
                        ps.tile([128, 2, 256], F32,
                                tag=(next_slot() if lead else "pq"), name="pq")
                    )
                pq = pq_box[0]
                for ko in (ko0, ko0 + 1):
                    st = ko == 0
                    sp = ko == 7 and not use_bias
                    nc.tensor.matmul(
                        pq[:], lhsT=hT_sb[:, ko, ssl], rhs=wq_sb[:, ko, :],
                        start=st, stop=sp,
                    )
                if ko0 == 6 and use_bias:
                    nc.tensor.matmul(pq[:], lhsT=ones1[:], rhs=brq_sb[:],
                                     start=False, stop=True)

            def q_sub_tail(sc, rms16, i, qros, pq):
                t12 = rt.tile([128, 2, 256], BF16, tag="t12")
                nc.vector.tensor_tensor(t12[:], pq[:], csq_sb[:, sc, :, :], AL.mult)
                qro = rop.tile([128, 4, 64], BF16, tag="qro")
                nc.gpsimd.tensor_tensor(
                    qro[:].rearrange("p h d -> p (h d)"), t12[:, 0, :], t12[:, 1, :],
                    AL.add,
                )
                qros.append(qro)
                sqq = rt.tile([128, 4, 64], BF16, tag="sqq")
                nc.gpsimd.scalar_tensor_tensor(
                    sqq[:], qro[:], 1.0 / 64.0, qro[:], AL.mult, AL.mult
                )
                nc.vector.tensor_reduce(rms16[:, 4 * i:4 * i + 4], sqq[:], AX.X, AL.add)

            def q_fin(ic, rms16, qros):
                rcq = rsqrt_batch(rms16, "rcq")
                for i in range(4):
                    sc = 4 * ic + i
                    qn = rt.tile([128, 4, 64], BF16, tag="qn")
                    nc.vector.tensor_tensor(
                        qn[:], qros[i][:],
                        rcq[:, 4 * i:4 * i + 4, None].to_broadcast((128, 4, 64)),
                        AL.mult,
                    )
                    nc.sync.dma_start_transpose(
                        out=qT[:, :, sc * 128:(sc + 1) * 128], in_=qn[:]
                    )

            def norm_half(h, oraw, box):
                # oraw: [128, 8, 128] psum, slices (s2, hd) at s2*4+hd, col 64 = denom
                rcp = stp.tile([128, 8], F32, tag="rcp")
                nc.vector.reciprocal(rcp[:], oraw[:, :, 64:65])
                for s2 in range(2):
                    onat = onp_.tile([128, 4, 64], BF16, tag="onat")
                    nc.vector.tensor_tensor(
                        onat[:], oraw[:, s2 * 4:(s2 + 1) * 4, 0:64],
                        rcp[:, s2 * 4:(s2 + 1) * 4, None].to_broadcast((128, 4, 64)),
                        AL.mult,
                    )
                    oTt = otp.tile([128, 2, 128], BF16, tag="oTt")
                    nc.sync.dma_start_transpose(out=oTt[:], in_=onat[:])
                    box.append(oTt)

            def outproj_sub(gc, oTt):
                ysb = yp.tile([128, HID], F32, tag="ysb")
                for ec in range(2):
                    py = ps.tile([128, 512], F32, tag="py")
                    for cc in range(2):
                        nc.tensor.matmul(
                            py[:], lhsT=oTt[:, cc, :],
                            rhs=wo_sb[:, cc, ec * 512:(ec + 1) * 512],
                            start=(cc == 0), stop=(cc == 1),
                        )
                    nc.vector.tensor_copy(ysb[:, ec * 512:(ec + 1) * 512], py[:])
                nc.sync.dma_start(out=y[gc], in_=ysb[:])

            # ---- lead-in: k/v for all chunks, then q for ic 0 --------
            for g in range(4):
                rmk4 = stp.tile([128, 4], F32, tag="rmk4")
                kros = []
                for i in range(4):
                    kv_sub(4 * g + i, rmk4, i, kros)
                kv_fin(g, rmk4, kros)
            rms16 = stp.tile([128, 16], F32, tag="rms16")
            qros = []
            for i in range(4):
                q_sub(i, rms16, i, qros, lead=True)
            q_fin(0, rms16, qros)

            # ---- attention: exp spine + drip-fed side work -----------
            # Two FIFO queues keep non-spine work out of the scores->exp
            # chain's way: wpv (PV sweeps + normalize/out_proj, ordered to
            # respect the single oraw psum slot) and wq (next block's q
            # projection chain).
            from collections import deque
            wpv, wq_ = deque(), deque()

            def pump(q, n):
                for _ in range(n):
                    if not q:
                        return
                    q.popleft()()

            def pv_group(oraw_box, pts, jc, subs):
                def run():
                    if not oraw_box:
                        oraw_box.append(
                            ps.tile([128, 8, 128], F32, tag="oraw", name="oraw")
                        )
                    oraw = oraw_box[0]
                    for s2, sub in enumerate(subs):
                        for hd in range(4):
                            nc.tensor.matmul(
                                oraw[:, s2 * 4 + hd, 0:65],
                                lhsT=pts[jc][:, hd, sub * 128:(sub + 1) * 128],
                                rhs=v_sb[:, jc, 0:65],
                                start=(jc == 0), stop=(jc == 15),
                            )
                return run

            state = {}  # per-ic boxes

            for ic in range(NIC):
                isl = slice(ic * 512, (ic + 1) * 512)
                pts = []
                state[ic] = dict(pts=pts, o0=[], o1=[], oT0=[], oT1=[])
                # previous block's second PV half + its finish
                if ic > 0:
                    pv = state[ic - 1]
                    for jc in range(16):
                        wpv.append(pv_group(pv["o1"], pv["pts"], jc, (2, 3)))
                    wpv.append(
                        (lambda p: lambda: norm_half(1, p["o1"][0], p["oT1"]))(pv)
                    )
                    wpv.append(
                        (lambda p, i: lambda: outproj_sub(i * 4 + 2, p["oT1"][0]))(pv, ic - 1)
                    )
                    wpv.append(
                        (lambda p, i: lambda: outproj_sub(i * 4 + 3, p["oT1"][1]))(pv, ic - 1)
                    )
                # next block's q projection chain
                if ic < 3:
                    nrms = stp.tile([128, 16], F32, tag="rms16")
                    nqros = []
                    for i in range(4):
                        sc = 4 * (ic + 1) + i
                        pq_box = []
                        for ko0 in (0, 2, 4, 6):
                            wq_.append(
                                (lambda s, b, k: lambda: q_sub_mm(s, b, k))(sc, pq_box, ko0)
                            )
                        wq_.append(
                            (lambda s, b, i2: lambda: q_sub_tail(s, nrms, i2, nqros, b[0]))(sc, pq_box, i)
                        )

                for jc in range(16):
                    pT_t = ptp.tile([128, 4, 512], BF16, tag="pT")
                    pts.append(pT_t)
                    if use_mask:
                        mkt = mp.tile([128, 512], F32, tag="mkt")
                        nc.sync.dma_start(out=mkt[:], in_=mk[jc][:, isl])
                    for pair in range(2):
                        pss = ps.tile([128, 2, 512], F32,
                                      tag=("scA" if pair == 0 else "scB"))
                        for hh in range(2):
                            rows = slice(64 * hh, 64 * hh + 64)
                            nc.tensor.matmul(
                                pss[:, hh, :],
                                lhsT=kT[rows, jc * 128:(jc + 1) * 128],
                                rhs=qT[rows, pair, isl],
                                start=True, stop=True,
                            )
                        if use_mask:
                            sm = mp.tile([128, 2, 512], F32, tag="sm")
                            nc.vector.scalar_tensor_tensor(
                                sm[:], pss[:], 0.125,
                                mkt[:, None, :].to_broadcast((128, 2, 512)),
                                AL.mult, AL.add,
                            )
                            nc.scalar.activation(
                                pT_t[:, 2 * pair:2 * pair + 2, :], sm[:], AF.Exp
                            )
                        else:
                            nc.scalar.activation(
                                pT_t[:, 2 * pair:2 * pair + 2, :], pss[:], AF.Exp,
                                scale=0.125,
                            )
                    # this block's first-half PV, lagged behind the spine
                    wpv.append(pv_group(state[ic]["o0"], pts, jc, (0, 1)))
                    pump(wpv, 4)
                    if ic < 3 and jc == 13:
                        pump(wq_, len(wq_))
                        q_fin(ic + 1, nrms, nqros)
                    else:
                        pump(wq_, 2)
                # drain this block's local work (h0 PV tail)
                pump(wpv, len(wpv))
                norm_half(0, state[ic]["o0"][0], state[ic]["oT0"])
                outproj_sub(ic * 4 + 0, state[ic]["oT0"][0])
                outproj_sub(ic * 4 + 1, state[ic]["oT0"][1])

            # tail: last block's second half
            pv = state[3]
            for jc in range(16):
                pv_group(pv["o1"], pv["pts"], jc, (2, 3))()
            norm_half(1, pv["o1"][0], pv["oT1"])
            outproj_sub(14, pv["oT1"][0])
            outproj_sub(15, pv["oT1"][1])

    nc.compile()
    return nc


def _get(use_mask: bool, use_bias: bool = False):
    key = (use_mask, use_bias)
    if key not in _cache:
        _cache[key] = _build(use_mask, use_bias)
    return _cache[key]


def _host_prep(hidden_state, attention_mask, Wq, bq, Wk, bk, Wv, bv, Wo,
               use_mask, use_bias):
    """Build the 8 per-core input maps."""
    half_q, half_k = HID // 2, (HKV * D) // 2  # 512, 128
    inv_q = ROPE_BASE ** (-np.arange(half_q, dtype=np.float64) / half_q)
    inv_k = ROPE_BASE ** (-np.arange(half_k, dtype=np.float64) / half_k)
    s_idx = np.arange(S, dtype=np.float64)
    ang_q = inv_q[:, None] * s_idx[None, :]  # [512, S] freq-major
    ang_k = inv_k[:, None] * s_idx[None, :]  # [128, S]
    cos_q, sin_q = np.cos(ang_q), np.sin(ang_q)
    cos_k, sin_k = np.cos(ang_k), np.sin(ang_k)

    in_maps = []
    for core in range(8):
        b, j = core // 4, core % 4
        own_q = np.arange(j * 256, (j + 1) * 256)
        par_q = own_q + 512 if j < 2 else own_q - 512
        fidx_q = own_q if j < 2 else own_q - 512
        sign = -1.0 if j < 2 else 1.0
        own_k = np.arange(j * 64, (j + 1) * 64)
        par_k = own_k + 128 if j < 2 else own_k - 128
        fidx_k = own_k if j < 2 else own_k - 128

        hTc = np.ascontiguousarray(hidden_state[b].T).astype(NB).reshape(8, 128, S)
        wq_c = np.concatenate([Wq[:, own_q], Wq[:, par_q]], axis=1)
        wq_c = wq_c.astype(NB).reshape(8, 128, QSEL)
        wk_c = np.concatenate([Wk[:, own_k], Wk[:, par_k]], axis=1)
        wk_c = wk_c.astype(NB).reshape(8, 128, 128)
        wv_c = Wv[:, own_k].astype(NB).reshape(8, 128, 64)
        wo_c = Wo[j * 256:(j + 1) * 256, :].astype(NB).reshape(2, 128, HID)
        # natural-layout cos/sin: [sc, seq128, {cos, signed sin}, feat]
        csq_c = np.stack(
            [cos_q[fidx_q].T, (sign * sin_q[fidx_q]).T], axis=1
        )  # [S, 2, 256]
        csq_c = csq_c.astype(NB).reshape(NSC, 128, 2, 256)
        csk_c = np.stack(
            [cos_k[fidx_k].T, (sign * sin_k[fidx_k]).T], axis=1
        )  # [S, 2, 64]
        csk_c = csk_c.astype(NB).reshape(NSC, 128, 2, 64)

        m = {
            "hT": hTc, "wq": wq_c, "wk": wk_c, "wv": wv_c, "wo": wo_c,
            "csq": csq_c, "csk": csk_c,
        }
        if use_mask:
            mT = np.ascontiguousarray(attention_mask[b].T).astype(np.float32)
            m["mk"] = mT.reshape(NSC, 128, S)
        if use_bias:
            m["brq"] = np.concatenate([bq[own_q], bq[par_q]]).astype(NB).reshape(1, QSEL)
            m["brk"] = np.concatenate([bk[own_k], bk[par_k]]).astype(NB).reshape(1, 128)
            m["brv"] = bv[own_k].astype(NB).reshape(1, 64)
        in_maps.append(m)
    return in_maps


def kernel(hidden_state, attention_mask, Wq, bq, Wk, bk, Wv, bv, Wo, bo):
    from concourse.bass_utils import run_bass_kernel_spmd

    hidden_state = np.asarray(hidden_state, dtype=np.float32)
    attention_mask = np.asarray(attention_mask, dtype=np.float32)
    Wq, bq = np.asarray(Wq, np.float32), np.asarray(bq, np.float32)
    Wk, bk = np.asarray(Wk, np.float32), np.asarray(bk, np.float32)
    Wv, bv = np.asarray(Wv, np.float32), np.asarray(bv, np.float32)
    Wo, bo = np.asarray(Wo, np.float32), np.asarray(bo, np.float32)
    use_mask = bool(np.any(attention_mask))
    use_bias = bool(np.any(bq) or np.any(bk) or np.any(bv))
    nc = _get(use_mask, use_bias)
    in_maps = _host_prep(
        hidden_state, attention_mask, Wq, bq, Wk, bk, Wv, bv, Wo,
        use_mask, use_bias,
    )
    res = run_bass_kernel_spmd(nc, in_maps, list(range(8)))
    out = np.zeros((B, S, HID), dtype=np.float32)
    for core in range(8):
        out[core // 4] += res.results[core]["y"].reshape(S, HID)
    out += bo[None, None, :]
    return out
